# revision 17
# baseline (speedup 1.0000x reference)
# Trainium2 Bass kernel for nn_DiffNet — transposed (feature-major) layout, v5.
#
# Math: with coef = (conv2_w @ conv1_w)[0] = (c0,c1,c2),
# bc = conv2_w@conv1_b + conv2_b, scale = RATE/batch_num, C* = scale*(c*,bc),
# each layer of the reference reduces exactly to
#   z = vi @ W.T;  vj = relu(z + b)
#   out = (1 + C2*s)*vj + C1*z + (C0*q + Cb*s)        s = sum(vi), q = sum(vi^2)
# The kernel computes out' = alpha*relu(P) + C1*P + delta with P = z + b:
# the extra C1*b it carries (and every term it induces downstream) is
# O(C1^2) ~ 1e-5 of the output — far below the error budget — so the exact
# m-offset folding of the reference implementation is dropped entirely.
#
# Layout: everything is feature-major.  P.T [nout, B] lands directly in PSUM
# from matmuls whose STATIONARY operand is a [128,128] fp16 weight chunk
# (full 128-column weight -> FWL, ~27ns/matmul), rhs = vi.T [128, 8] fp16.
# alpha/delta are accumulated broadcast-down-partitions by the PE itself:
#   alphaB [128,8] = sum_k (C2*ones128x128) @ viT_k + ones-rank1(1.0)
#   deltaB [128,8] = sum_k (Cb*ones) @ viT_k + sum_k (C0*ones) @ sq_k
# using three host-uploaded constant-times-ones fp16 matrices that ride in
# the wall.  The epilogue is 5 ops on [128, 8*nch] tiles, all with real
# data dependencies only (no cross-engine row/broadcast round-trip, which
# the tile scheduler's simulated-order semaphores would serialize badly).
#
# DMA: one fp16 wall (xT | cones | L1 | L2 | L3) in 4 slices split across the
# sync and scalar HWDGE rings, plus a single 1-descriptor constant row.
#
# Sharding: data-parallel over batch (64 -> 8 rows/core), weights replicated,
# zero collectives.  Host transposes the [128, 16] per-core result back.

import numpy as np

RATE = 0.01
B, IN, H1, H2, OUT = 64, 1024, 512, 512, 256
NCORES = 8
BL = B // NCORES
P128 = 128

NK = [IN // P128, H1 // P128, H2 // P128]    # 8, 4, 4
NCH = [H1 // P128, H2 // P128, OUT // P128]  # 4, 4, 2

# wall fp16 [128, 7616]: xT | c2ones | cbones | c0ones | L1 | L2 | L3
XT_OFF = 0
XT_LEN = NK[0] * BL  # 64
CONES_OFF = XT_LEN   # 3 x [128,128] constant*ones matrices
WOFF = [CONES_OFF + 3 * P128, CONES_OFF + 3 * P128 + 4096,
        CONES_OFF + 3 * P128 + 6144]
W_LEN = WOFF[2] + 1024  # 7616
WSLICES = [
    (0, WOFF[0]),                  # xT + cones  (sync)
    (WOFF[0], WOFF[0] + 2048),     # L1 c0,c1    (sync)
    (WOFF[0] + 2048, WOFF[1]),     # L1 c2,c3    (sync)
    (WOFF[1], WOFF[2]),            # L2          (scalar)
    (WOFF[2], W_LEN),              # L3          (scalar)
]

# bh4 fp16 [4, 417]: per-layer bias as [nch, 128] rows | blockdiag(4x32) | C1
BH_OFF4 = [0, P128, 2 * P128]
BD_OFF = 3 * P128
C1_OFF4 = BD_OFF + 32
BH4_LEN = C1_OFF4 + 1

N_WARMUP = 32

_NC_CACHE = {}


def _build_nc():
    import concourse.bacc as bacc
    import concourse.mybir as mybir
    import concourse.tile as tile
    from concourse.bass import AP

    fp32 = mybir.dt.float32
    fp16 = mybir.dt.float16
    AF = mybir.ActivationFunctionType
    ALU = mybir.AluOpType

    nc = bacc.Bacc("TRN2", target_bir_lowering=False, debug=False)

    w_t = nc.dram_tensor("wall", [P128, W_LEN], fp16, kind="ExternalInput")
    bh_t = nc.dram_tensor("bh4", [4, BH4_LEN], fp16, kind="ExternalInput")
    out_t = nc.dram_tensor("outT", [P128, 2 * BL], fp32, kind="ExternalOutput")

    with tile.TileContext(nc) as tc:
        with (
            tc.tile_pool(name="wp", bufs=1) as wp,
            tc.tile_pool(name="ap", bufs=1) as ap_,
            tc.tile_pool(name="xp", bufs=1, space="PSUM") as xp,
            tc.tile_pool(name="pp", bufs=2, space="PSUM") as pp,
            tc.tile_pool(name="bp", bufs=2, space="PSUM") as bp,
        ):
            # --- DMAs split across both HWDGE rings; const row early ---
            wseg = []

            def wdma(eng, i):
                lo, hi = WSLICES[i]
                t = wp.tile([P128, hi - lo], fp16, tag=f"w{i}")
                eng.dma_start(t[:], w_t[:, lo:hi])
                wseg.append((t, lo))

            bhr = ap_.tile([4, BH4_LEN], fp16, tag="bhr")
            nc.sync.dma_start(bhr[:], bh_t[:])
            wdma(nc.sync, 0)
            wdma(nc.sync, 1)
            wdma(nc.sync, 2)
            wdma(nc.scalar, 3)
            wdma(nc.scalar, 4)

            def wall(lo, n):
                for t, off in wseg:
                    if off <= lo and lo + n <= off + t.shape[1]:
                        return t[:, lo - off : lo - off + n]
                raise AssertionError("bad wall slice")

            # --- small on-device constants ---
            junk_a = wp.tile([BL, BL], fp16, tag="junk_a")
            junk_w = wp.tile([BL, 64], fp16, tag="junk_w")
            nc.gpsimd.memset(junk_a[:], 0.0)
            nc.gpsimd.memset(junk_w[:], 0.0)
            ones128 = wp.tile([1, P128], fp16, tag="ones128")
            nc.vector.memset(ones128[:], 1.0)
            o8 = wp.tile([1, BL], fp16, tag="o8")
            nc.vector.memset(o8[:], 1.0)

            # PE warm-up (HAM clock gate) while the weight DMA streams
            warm = xp.tile([BL, 64], fp32, tag="warm")
            for _ in range(N_WARMUP):
                nc.tensor.matmul(warm[:], junk_a[:], junk_w[:], start=True, stop=True)

            # C1 broadcast down partitions -> [128,1] fp32 scalar operand
            ccp = xp.tile([P128, 1], fp32, tag="ccol")
            nc.tensor.matmul(
                ccp[:], ones128[:], bhr[0:1, C1_OFF4 : C1_OFF4 + 1],
                start=True, stop=True,
            )
            c1c = ap_.tile([P128, 1], fp32, tag="c1c")
            nc.vector.tensor_copy(out=c1c[:], in_=ccp[:])

            def bcast_ap(t, nch):
                """[128, 8] tile slice -> [128, nch, 8] 0-stride broadcast."""
                return AP(t.tensor, t.offset, [t.ap[0], [0, nch], t.ap[1]])

            def layer(l, viT, sq):
                nk, nch = NK[l], NCH[l]
                ncol = nch * BL
                nh = max(nch // 2, 1)   # chunks per half
                hc = nh * BL            # columns per half

                # alphaB/deltaB [128, 8] each, accumulated purely on the PE
                def bc_mms():
                    for k in range(nk):
                        nc.tensor.matmul(
                            bc[:, 0:BL], wall(CONES_OFF, P128),
                            viT[:, k * BL : (k + 1) * BL],
                            start=(k == 0), stop=False,
                        )
                    nc.tensor.matmul(
                        bc[:, 0:BL], ones128[:], o8[:], start=False, stop=True
                    )
                    for k in range(nk):
                        nc.tensor.matmul(
                            bc[:, BL : 2 * BL], wall(CONES_OFF + P128, P128),
                            viT[:, k * BL : (k + 1) * BL],
                            start=(k == 0), stop=False,
                        )
                    for k in range(nk):
                        nc.tensor.matmul(
                            bc[:, BL : 2 * BL], wall(CONES_OFF + 2 * P128, P128),
                            sq[:, k * BL : (k + 1) * BL],
                            start=False, stop=(k == nk - 1),
                        )

                bc = bp.tile([P128, 2 * BL], fp32, tag="bc")
                if l == 0:
                    # L1 weights are DMA-gated; bc overlaps the wait
                    bc_mms()

                # P.T: one accumulation group over the whole tile; bias for
                # every chunk lands in a single block-diagonal matmul
                Pt = pp.tile([P128, ncol], fp32, tag="P")
                o = ap_.tile([P128, ncol], fp32 if l == 2 else fp16, tag=f"o{l}")
                sqn = (
                    None if l == 2
                    else ap_.tile([P128, ncol], fp16, tag=f"sqn{l}")
                )
                for c in range(nch):
                    for k in range(nk):
                        nc.tensor.matmul(
                            Pt[:, c * BL : (c + 1) * BL],
                            wall(WOFF[l] + (c * nk + k) * P128, P128),
                            viT[:, k * BL : (k + 1) * BL],
                            start=(c == 0 and k == 0), stop=False,
                            skip_group_check=True,
                        )
                nc.tensor.matmul(
                    Pt[:],
                    bhr[0:nch, BH_OFF4[l] : BH_OFF4[l] + P128],
                    bhr[0:nch, BD_OFF : BD_OFF + ncol],
                    start=False, stop=True, skip_group_check=True,
                )
                if l > 0:
                    bc_mms()

                R = ap_.tile([P128, ncol], fp32, tag=f"R{l}")
                nc.scalar.activation(out=R[:], in_=Pt[:], func=AF.Relu)
                t2 = ap_.tile([P128, ncol], fp32, tag=f"t2{l}")
                nc.vector.tensor_scalar(t2[:], Pt[:], c1c[:], None, ALU.mult)
                t3 = ap_.tile([P128, ncol], fp32, tag=f"t3{l}")
                nc.vector.tensor_tensor(
                    t3[:], t2[:], bcast_ap(bc[:, BL : 2 * BL], nch), ALU.add
                )
                t4 = ap_.tile([P128, ncol], fp32, tag=f"t4{l}")
                nc.vector.tensor_tensor(
                    t4[:], R[:], bcast_ap(bc[:, 0:BL], nch), ALU.mult
                )
                if l == 2:
                    nc.vector.tensor_tensor(
                        o[:, 0:BL], t3[:, 0:BL], t4[:, 0:BL], ALU.add
                    )
                    nc.sync.dma_start(
                        out_t[:, 0:BL], o[:, 0:BL], single_packet=True
                    )
                    nc.vector.tensor_tensor(
                        o[:, BL : 2 * BL], t3[:, BL : 2 * BL],
                        t4[:, BL : 2 * BL], ALU.add,
                    )
                    nc.scalar.dma_start(
                        out_t[:, BL : 2 * BL], o[:, BL : 2 * BL],
                        single_packet=True,
                    )
                else:
                    nc.vector.tensor_tensor(o[:], t3[:], t4[:], ALU.add)
                    nc.vector.tensor_tensor(sqn[:], o[:], o[:], ALU.mult)
                return o, sqn

            xT = wall(XT_OFF, XT_LEN)
            sq1 = ap_.tile([P128, XT_LEN], fp16, tag="sq1")
            nc.scalar.activation(out=sq1[:], in_=xT, func=AF.Square)
            o1, sq2 = layer(0, xT, sq1)
            o2, sq3 = layer(1, o1, sq2)
            o3, _ = layer(2, o2, sq3)

    nc.compile()
    return nc


def get_nc():
    if "nc" not in _NC_CACHE:
        _NC_CACHE["nc"] = _build_nc()
    return _NC_CACHE["nc"]


def _wchunks(Wt, nk, nch):
    """[in, out] -> [128, nch*nk*128]: chunk (k, c) at col (c*nk+k)*128."""
    return (
        Wt.reshape(nk, P128, nch, P128)
        .transpose(1, 2, 0, 3)
        .reshape(P128, nch * nk * P128)
    ).astype(np.float16)


def host_prep(x, fc1_w, fc1_b, fc2_w, fc2_b, fc3_w, fc3_b,
              conv1_w, conv1_b, conv2_w, conv2_b, batch_num):
    f32, f16, f64 = np.float32, np.float16, np.float64
    x = np.asarray(x, f32)
    ws = [np.asarray(fc1_w, f32), np.asarray(fc2_w, f32), np.asarray(fc3_w, f32)]
    bs = [np.asarray(fc1_b, f32), np.asarray(fc2_b, f32), np.asarray(fc3_b, f32)]

    bn = float(np.asarray(batch_num).item())
    scale = RATE / bn
    coef = (np.asarray(conv2_w, f64) @ np.asarray(conv1_w, f64))[0]
    bcv = float(
        (np.asarray(conv2_w, f64) @ np.asarray(conv1_b, f64))[0]
        + np.asarray(conv2_b, f64)[0]
    )
    C0, C1, C2 = (scale * coef).astype(f64)
    Cb = scale * bcv

    bh4 = np.zeros((4, BH4_LEN), f16)
    for l in range(3):
        nch = NCH[l]
        bh4[0:nch, BH_OFF4[l] : BH_OFF4[l] + P128] = (
            bs[l].astype(f16).reshape(nch, P128)
        )
    for c in range(4):
        bh4[c, BD_OFF + c * BL : BD_OFF + (c + 1) * BL] = 1.0
    bh4[0, C1_OFF4] = f16(C1)

    wall_base = np.empty((P128, W_LEN), f16)
    for i, cv in enumerate((C2, Cb, C0)):
        wall_base[:, CONES_OFF + i * P128 : CONES_OFF + (i + 1) * P128] = f16(cv)
    for l in range(3):
        wall_base[:, WOFF[l] : WOFF[l] + NCH[l] * NK[l] * P128] = _wchunks(
            ws[l].T, NK[l], NCH[l]
        )

    in_maps = []
    for k in range(NCORES):
        xk = x[k * BL : (k + 1) * BL]
        wall = wall_base.copy()
        wall[:, XT_OFF : XT_OFF + XT_LEN] = (
            xk.T.reshape(NK[0], P128, BL).transpose(1, 0, 2).reshape(P128, XT_LEN)
        ).astype(f16)
        in_maps.append({"wall": wall, "bh4": bh4})
    return in_maps


def _unshard(outT):
    """[128, 16] -> [8, 256]: out[b, c*128+p] = outT[p, c*8+b]."""
    return np.ascontiguousarray(
        outT.reshape(P128, 2, BL).transpose(2, 1, 0).reshape(BL, OUT), dtype=np.float32
    )


def kernel(**inputs):
    from concourse.bass_utils import run_bass_kernel_spmd

    nc = get_nc()
    in_maps = host_prep(**inputs)
    res = run_bass_kernel_spmd(nc, in_maps, core_ids=list(range(NCORES)))
    out = np.concatenate(
        [_unshard(res.results[k]["outT"]) for k in range(NCORES)], axis=0
    )
    return np.ascontiguousarray(out, dtype=np.float32)


# revision 19
# speedup vs baseline: 1.0897x; 1.0897x over previous
# Trainium2 Bass kernel for nn_DiffNet — transposed (feature-major) layout, v5.
#
# Math: with coef = (conv2_w @ conv1_w)[0] = (c0,c1,c2),
# bc = conv2_w@conv1_b + conv2_b, scale = RATE/batch_num, C* = scale*(c*,bc),
# each layer of the reference reduces exactly to
#   z = vi @ W.T;  vj = relu(z + b)
#   out = (1 + C2*s)*vj + C1*z + (C0*q + Cb*s)        s = sum(vi), q = sum(vi^2)
# The kernel computes out' = alpha*relu(P) + C1*P + delta with P = z + b:
# the extra C1*b it carries (and every term it induces downstream) is
# O(C1^2) ~ 1e-5 of the output — far below the error budget — so the exact
# m-offset folding of the reference implementation is dropped entirely.
#
# Layout: everything is feature-major.  P.T [nout, B] lands directly in PSUM
# from matmuls whose STATIONARY operand is a [128,128] fp16 weight chunk
# (full 128-column weight -> FWL, ~27ns/matmul), rhs = vi.T [128, 8] fp16.
# alpha/delta are accumulated broadcast-down-partitions by the PE itself:
#   alphaB [128,8] = sum_k (C2*ones128x128) @ viT_k + ones-rank1(1.0)
#   deltaB [128,8] = sum_k (Cb*ones) @ viT_k + sum_k (C0*ones) @ sq_k
# using three host-uploaded constant-times-ones fp16 matrices that ride in
# the wall.  The epilogue is 5 ops on [128, 8*nch] tiles, all with real
# data dependencies only (no cross-engine row/broadcast round-trip, which
# the tile scheduler's simulated-order semaphores would serialize badly).
#
# DMA: one fp16 wall (xT | cones | L1 | L2 | L3) in 4 slices split across the
# sync and scalar HWDGE rings, plus a single 1-descriptor constant row.
#
# Sharding: data-parallel over batch (64 -> 8 rows/core), weights replicated,
# zero collectives.  Host transposes the [128, 16] per-core result back.

import numpy as np

RATE = 0.01
B, IN, H1, H2, OUT = 64, 1024, 512, 512, 256
NCORES = 8
BL = B // NCORES
P128 = 128

NK = [IN // P128, H1 // P128, H2 // P128]    # 8, 4, 4
NCH = [H1 // P128, H2 // P128, OUT // P128]  # 4, 4, 2

# wall fp16 [128, 7616]: xT | c2ones | cbones | c0ones | L1 | L2 | L3
XT_OFF = 0
XT_LEN = NK[0] * BL  # 64
CONES_OFF = XT_LEN   # 3 x [128,128] constant*ones matrices
WOFF = [CONES_OFF + 3 * P128, CONES_OFF + 3 * P128 + 4096,
        CONES_OFF + 3 * P128 + 6144]
W_LEN = WOFF[2] + 1024  # 7616
WSLICES = [
    (0, WOFF[0]),        # xT + cones          (sync)
    (WOFF[0], WOFF[1]),  # all of L1, 8KB/desc (sync)
    (WOFF[1], W_LEN),    # L2 + L3, 6KB/desc   (scalar)
]

# bh4 fp16 [4, 417]: per-layer bias as [nch, 128] rows | blockdiag(4x32) | C1
BH_OFF4 = [0, P128, 2 * P128]
BD_OFF = 3 * P128
C1_OFF4 = BD_OFF + 32
BH4_LEN = C1_OFF4 + 1

N_WARMUP = 32

_NC_CACHE = {}


def _build_nc():
    import concourse.bacc as bacc
    import concourse.mybir as mybir
    import concourse.tile as tile
    from concourse.bass import AP

    fp32 = mybir.dt.float32
    fp16 = mybir.dt.float16
    AF = mybir.ActivationFunctionType
    ALU = mybir.AluOpType

    nc = bacc.Bacc("TRN2", target_bir_lowering=False, debug=False)

    w_t = nc.dram_tensor("wall", [P128, W_LEN], fp16, kind="ExternalInput")
    bh_t = nc.dram_tensor("bh4", [4, BH4_LEN], fp16, kind="ExternalInput")
    out_t = nc.dram_tensor("outT", [P128, 2 * BL], fp32, kind="ExternalOutput")

    with tile.TileContext(nc) as tc:
        with (
            tc.tile_pool(name="wp", bufs=1) as wp,
            tc.tile_pool(name="ap", bufs=1) as ap_,
            tc.tile_pool(name="xp", bufs=1, space="PSUM") as xp,
            tc.tile_pool(name="pp", bufs=2, space="PSUM") as pp,
            tc.tile_pool(name="bp", bufs=2, space="PSUM") as bp,
        ):
            # --- DMAs split across both HWDGE rings; const row early ---
            wseg = []

            def wdma(eng, i):
                lo, hi = WSLICES[i]
                t = wp.tile([P128, hi - lo], fp16, tag=f"w{i}")
                eng.dma_start(t[:], w_t[:, lo:hi])
                wseg.append((t, lo))

            wdma(nc.sync, 0)
            bhr = ap_.tile([4, BH4_LEN], fp16, tag="bhr")
            nc.scalar.dma_start(bhr[:], bh_t[:])
            wdma(nc.sync, 1)
            wdma(nc.scalar, 2)

            def wall(lo, n):
                for t, off in wseg:
                    if off <= lo and lo + n <= off + t.shape[1]:
                        return t[:, lo - off : lo - off + n]
                raise AssertionError("bad wall slice")

            # --- small on-device constants ---
            junk_a = wp.tile([BL, BL], fp16, tag="junk_a")
            junk_w = wp.tile([BL, 64], fp16, tag="junk_w")
            nc.gpsimd.memset(junk_a[:], 0.0)
            nc.gpsimd.memset(junk_w[:], 0.0)
            ones128 = wp.tile([1, P128], fp16, tag="ones128")
            nc.vector.memset(ones128[:], 1.0)
            o8 = wp.tile([1, BL], fp16, tag="o8")
            nc.vector.memset(o8[:], 1.0)

            # PE warm-up (HAM clock gate) while the weight DMA streams
            warm = xp.tile([BL, 64], fp32, tag="warm")
            for _ in range(N_WARMUP):
                nc.tensor.matmul(warm[:], junk_a[:], junk_w[:], start=True, stop=True)

            # C1 broadcast down partitions -> [128,1] fp32 scalar operand
            ccp = xp.tile([P128, 1], fp32, tag="ccol")
            nc.tensor.matmul(
                ccp[:], ones128[:], bhr[0:1, C1_OFF4 : C1_OFF4 + 1],
                start=True, stop=True,
            )
            c1c = ap_.tile([P128, 1], fp32, tag="c1c")
            nc.vector.tensor_copy(out=c1c[:], in_=ccp[:])

            def bcast_ap(t, nch):
                """[128, 8] tile slice -> [128, nch, 8] 0-stride broadcast."""
                return AP(t.tensor, t.offset, [t.ap[0], [0, nch], t.ap[1]])

            def layer(l, viT, sq):
                nk, nch = NK[l], NCH[l]
                ncol = nch * BL
                nh = max(nch // 2, 1)   # chunks per half
                hc = nh * BL            # columns per half

                # alphaB/deltaB [128, 8] each, accumulated purely on the PE
                def bc_mms():
                    for k in range(nk):
                        nc.tensor.matmul(
                            bc[:, 0:BL], wall(CONES_OFF, P128),
                            viT[:, k * BL : (k + 1) * BL],
                            start=(k == 0), stop=False,
                        )
                    nc.tensor.matmul(
                        bc[:, 0:BL], ones128[:], o8[:], start=False, stop=True
                    )
                    for k in range(nk):
                        nc.tensor.matmul(
                            bc[:, BL : 2 * BL], wall(CONES_OFF + P128, P128),
                            viT[:, k * BL : (k + 1) * BL],
                            start=(k == 0), stop=False,
                        )
                    for k in range(nk):
                        nc.tensor.matmul(
                            bc[:, BL : 2 * BL], wall(CONES_OFF + 2 * P128, P128),
                            sq[:, k * BL : (k + 1) * BL],
                            start=False, stop=(k == nk - 1),
                        )

                bc = bp.tile([P128, 2 * BL], fp32, tag="bc")
                bc_mms()

                # P.T: one accumulation group over the whole tile; bias for
                # every chunk lands in a single block-diagonal matmul
                Pt = pp.tile([P128, ncol], fp32, tag="P")
                o = ap_.tile([P128, ncol], fp32 if l == 2 else fp16, tag=f"o{l}")
                sqn = (
                    None if l == 2
                    else ap_.tile([P128, ncol], fp16, tag=f"sqn{l}")
                )
                for c in range(nch):
                    for k in range(nk):
                        nc.tensor.matmul(
                            Pt[:, c * BL : (c + 1) * BL],
                            wall(WOFF[l] + (c * nk + k) * P128, P128),
                            viT[:, k * BL : (k + 1) * BL],
                            start=(c == 0 and k == 0), stop=False,
                            skip_group_check=True,
                        )
                nc.tensor.matmul(
                    Pt[:],
                    bhr[0:nch, BH_OFF4[l] : BH_OFF4[l] + P128],
                    bhr[0:nch, BD_OFF : BD_OFF + ncol],
                    start=False, stop=True, skip_group_check=True,
                )
                R = ap_.tile([P128, ncol], fp32, tag=f"R{l}")
                nc.scalar.activation(out=R[:], in_=Pt[:], func=AF.Relu)
                t2 = ap_.tile([P128, ncol], fp32, tag=f"t2{l}")
                nc.vector.tensor_scalar(t2[:], Pt[:], c1c[:], None, ALU.mult)
                t3 = ap_.tile([P128, ncol], fp32, tag=f"t3{l}")
                nc.vector.tensor_tensor(
                    t3[:], t2[:], bcast_ap(bc[:, BL : 2 * BL], nch), ALU.add
                )
                t4 = ap_.tile([P128, ncol], fp32, tag=f"t4{l}")
                nc.vector.tensor_tensor(
                    t4[:], R[:], bcast_ap(bc[:, 0:BL], nch), ALU.mult
                )
                if l == 2:
                    nc.vector.tensor_tensor(
                        o[:, 0:BL], t3[:, 0:BL], t4[:, 0:BL], ALU.add
                    )
                    nc.sync.dma_start(
                        out_t[:, 0:BL], o[:, 0:BL], single_packet=True
                    )
                    nc.vector.tensor_tensor(
                        o[:, BL : 2 * BL], t3[:, BL : 2 * BL],
                        t4[:, BL : 2 * BL], ALU.add,
                    )
                    nc.scalar.dma_start(
                        out_t[:, BL : 2 * BL], o[:, BL : 2 * BL],
                        single_packet=True,
                    )
                else:
                    nc.vector.tensor_tensor(o[:], t3[:], t4[:], ALU.add)
                    nc.vector.tensor_tensor(sqn[:], o[:], o[:], ALU.mult)
                return o, sqn

            xT = wall(XT_OFF, XT_LEN)
            sq1 = ap_.tile([P128, XT_LEN], fp16, tag="sq1")
            nc.scalar.activation(out=sq1[:], in_=xT, func=AF.Square)
            o1, sq2 = layer(0, xT, sq1)
            o2, sq3 = layer(1, o1, sq2)
            o3, _ = layer(2, o2, sq3)

    nc.compile()
    return nc


def get_nc():
    if "nc" not in _NC_CACHE:
        _NC_CACHE["nc"] = _build_nc()
    return _NC_CACHE["nc"]


def _wchunks(Wt, nk, nch):
    """[in, out] -> [128, nch*nk*128]: chunk (k, c) at col (c*nk+k)*128."""
    return (
        Wt.reshape(nk, P128, nch, P128)
        .transpose(1, 2, 0, 3)
        .reshape(P128, nch * nk * P128)
    ).astype(np.float16)


def host_prep(x, fc1_w, fc1_b, fc2_w, fc2_b, fc3_w, fc3_b,
              conv1_w, conv1_b, conv2_w, conv2_b, batch_num):
    f32, f16, f64 = np.float32, np.float16, np.float64
    x = np.asarray(x, f32)
    ws = [np.asarray(fc1_w, f32), np.asarray(fc2_w, f32), np.asarray(fc3_w, f32)]
    bs = [np.asarray(fc1_b, f32), np.asarray(fc2_b, f32), np.asarray(fc3_b, f32)]

    bn = float(np.asarray(batch_num).item())
    scale = RATE / bn
    coef = (np.asarray(conv2_w, f64) @ np.asarray(conv1_w, f64))[0]
    bcv = float(
        (np.asarray(conv2_w, f64) @ np.asarray(conv1_b, f64))[0]
        + np.asarray(conv2_b, f64)[0]
    )
    C0, C1, C2 = (scale * coef).astype(f64)
    Cb = scale * bcv

    bh4 = np.zeros((4, BH4_LEN), f16)
    for l in range(3):
        nch = NCH[l]
        bh4[0:nch, BH_OFF4[l] : BH_OFF4[l] + P128] = (
            bs[l].astype(f16).reshape(nch, P128)
        )
    for c in range(4):
        bh4[c, BD_OFF + c * BL : BD_OFF + (c + 1) * BL] = 1.0
    bh4[0, C1_OFF4] = f16(C1)

    wall_base = np.empty((P128, W_LEN), f16)
    for i, cv in enumerate((C2, Cb, C0)):
        wall_base[:, CONES_OFF + i * P128 : CONES_OFF + (i + 1) * P128] = f16(cv)
    for l in range(3):
        wall_base[:, WOFF[l] : WOFF[l] + NCH[l] * NK[l] * P128] = _wchunks(
            ws[l].T, NK[l], NCH[l]
        )

    in_maps = []
    for k in range(NCORES):
        xk = x[k * BL : (k + 1) * BL]
        wall = wall_base.copy()
        wall[:, XT_OFF : XT_OFF + XT_LEN] = (
            xk.T.reshape(NK[0], P128, BL).transpose(1, 0, 2).reshape(P128, XT_LEN)
        ).astype(f16)
        in_maps.append({"wall": wall, "bh4": bh4})
    return in_maps


def _unshard(outT):
    """[128, 16] -> [8, 256]: out[b, c*128+p] = outT[p, c*8+b]."""
    return np.ascontiguousarray(
        outT.reshape(P128, 2, BL).transpose(2, 1, 0).reshape(BL, OUT), dtype=np.float32
    )


def kernel(**inputs):
    from concourse.bass_utils import run_bass_kernel_spmd

    nc = get_nc()
    in_maps = host_prep(**inputs)
    res = run_bass_kernel_spmd(nc, in_maps, core_ids=list(range(NCORES)))
    out = np.concatenate(
        [_unshard(res.results[k]["outT"]) for k in range(NCORES)], axis=0
    )
    return np.ascontiguousarray(out, dtype=np.float32)
